# revision 4
# baseline (speedup 1.0000x reference)
"""DTCWT level-1 biorthogonal layer (near_sym_a) on 8 Trainium2 NeuronCores.

Math: reference computes, per (batch, channel) 256x256 image X:
    Lo = row_h0(X); Hi = row_h1(X)            (row filter = along W, symmetric pad)
    LoLo = col_h0(Lo)
    LoHi = col_h1(Lo); HiLo = col_h0(Hi); HiHi = col_h1(Hi)   (col = along H)
    q2c on {LoHi, HiHi, HiLo} (scaled by 1/sqrt(2)) -> 6 complex subbands.

Row/col filters commute, so we evaluate col-first:
    V_f = col_hf(X), then <out> = row_hg(V_f).
Both 1-D filters become banded 256x256 matrices (symmetric padding folded in).

Kernel dataflow per image (per core; batch is sharded 1 batch/core):
  stage 1 (PE): psum_v[m][w, :] = sum_h X[h, w+128m] * [A0|A1][h, :]
     - lhsT = X chunk (stationary) => output arrives TRANSPOSED [W, H] for free
     - rhs = concat of both col filter band matrices -> [128, 512] psum
  ACT copy psum_v -> SBUF (v_sb, float32r)
  stage 2 (PE): for f, h-parity p: psum2[f][p] = sum_w V_f[w, 2j+p] * [B|B'][w, :]
     - lhsT = stride-2 columns of v_sb => h-parity split for q2c comes free
     - rhs sections: f=0 -> [LoLo (unscaled) | HiLo*s], f=1 -> [LoHi*s | HiHi*s]
  ACT copies: odd-parity psums -> SBUF, LoLo-even -> SBUF
  DVE q2c: a,b from even psum (direct), c,d from odd SBUF copies; +/- only
  DMA out: LoLo rows interleaved; 12 subband tiles 64KB contiguous each.

Matmuls run in float32r (TF32-ish reduced precision fp32, ~2e-4 rel err).
"""

import numpy as np

MM_MODE = "f32r"  # "f32r" | "f32" | "bf16"

N_CORES = 8
B, C, H, W = 8, 64, 256, 256
NIMG = C  # images per core (one batch per core)


def _conv_mat(h, n):
    """M such that (symmetric-pad correlate1d(x, h)) == M @ x, matching
    jnp.pad(mode='symmetric') + VALID conv_general_dilated (no kernel flip)."""
    L = len(h)
    m = L // 2
    M = np.zeros((n, n), dtype=np.float64)
    for i in range(n):
        for t in range(L):
            j = i + t - m
            if j < 0:
                j = -1 - j
            if j >= n:
                j = 2 * n - 1 - j
            M[i, j] += float(h[t])
    return M


def _build_consts(h0o, h1o):
    A0 = _conv_mat(np.asarray(h0o, dtype=np.float64), H).T  # [h_in, h_out]
    A1 = _conv_mat(np.asarray(h1o, dtype=np.float64), H).T
    s = 1.0 / np.sqrt(2.0)
    R1 = np.concatenate([A0, A1], axis=1)              # [256, 512] col-filter stage
    R2a = np.concatenate([A0, A1 * s], axis=1)         # f=0: [LoLo | HiLo*s]
    R2b = np.concatenate([A0 * s, A1 * s], axis=1)     # f=1: [LoHi*s | HiHi*s]
    f32 = lambda M: np.ascontiguousarray(M.reshape(2, 128, 512).astype(np.float32))
    return f32(R1), f32(R2a), f32(R2b)


def _build_nc(repeat=1):
    import concourse.bass as bass
    import concourse.mybir as mybir
    from concourse import bacc
    from concourse.tile import TileContext

    if MM_MODE == "f32r":
        mdt = mybir.dt.float32r
    elif MM_MODE == "bf16":
        mdt = mybir.dt.bfloat16
    else:
        mdt = mybir.dt.float32
    f32 = mybir.dt.float32

    nc = bacc.Bacc("TRN2", target_bir_lowering=False, debug=False, num_devices=N_CORES)
    x_d = nc.declare_dram_parameter("x", [NIMG, H, W], mdt, isOutput=False)
    r1_d = nc.declare_dram_parameter("r1", [2, 128, 512], mdt, isOutput=False)
    r2a_d = nc.declare_dram_parameter("r2a", [2, 128, 512], mdt, isOutput=False)
    r2b_d = nc.declare_dram_parameter("r2b", [2, 128, 512], mdt, isOutput=False)
    lolo_d = nc.declare_dram_parameter("lolo", [NIMG, H, W], f32, isOutput=True)
    yhr_d = nc.declare_dram_parameter("yhr", [NIMG, 6, 128, 128], f32, isOutput=True)
    yhi_d = nc.declare_dram_parameter("yhi", [NIMG, 6, 128, 128], f32, isOutput=True)

    with TileContext(nc) as tc:
        with (
            tc.tile_pool(name="const", bufs=1) as cpool,
            tc.tile_pool(name="xin", bufs=4) as xpool,
            tc.tile_pool(name="vsb", bufs=4) as vpool,
            tc.tile_pool(name="osb", bufs=4) as opool,
            tc.tile_pool(name="yout", bufs=14) as ypool,
            tc.tile_pool(name="pv", bufs=4, space="PSUM") as pvpool,
            tc.tile_pool(name="p2", bufs=4, space="PSUM") as p2pool,
        ):
            r1_t = cpool.tile([128, 2, 512], mdt)
            r2a_t = cpool.tile([128, 2, 512], mdt)
            r2b_t = cpool.tile([128, 2, 512], mdt)
            for k in range(2):
                nc.sync.dma_start(out=r1_t[:, k, :], in_=r1_d[k])
                nc.sync.dma_start(out=r2a_t[:, k, :], in_=r2a_d[k])
                nc.sync.dma_start(out=r2b_t[:, k, :], in_=r2b_d[k])
            r2_t = [r2a_t, r2b_t]

            with tc.For_i(0, repeat, 1):
                _image_loop(nc, tc, bass, mybir, mdt, f32,
                            x_d, lolo_d, yhr_d, yhi_d, r1_t, r2_t,
                            xpool, vpool, opool, ypool, pvpool, p2pool)
    nc.compile()
    return nc


def _image_loop(nc, tc, bass, mybir, mdt, f32, x_d, lolo_d, yhr_d, yhi_d,
                r1_t, r2_t, xpool, vpool, opool, ypool, pvpool, p2pool):
    if True:
            for c in range(NIMG):
                # ---- load image c: [h, k, w] with k = H/128 chunk
                xt = xpool.tile([128, 2, W], mdt, tag="x")
                for k in range(2):
                    nc.sync.dma_start(out=xt[:, k, :], in_=x_d[c, k * 128:(k + 1) * 128, :])

                # ---- stage 1: psum_v[m] [w_local, [V0|V1] h_out] for W chunk m
                pv = []
                for m in range(2):
                    t = pvpool.tile([128, 512], f32, tag="pv")
                    for k in range(2):
                        nc.tensor.matmul(
                            t[:], lhsT=xt[:, k, m * 128:(m + 1) * 128],
                            rhs=r1_t[:, k, :], start=(k == 0), stop=(k == 1),
                        )
                    pv.append(t)

                # ---- V to SBUF (rounded to matmul dtype for stage-2 lhsT)
                vsb = []
                for m in range(2):
                    t = vpool.tile([128, 512], mdt, tag="v")
                    nc.scalar.copy(t[:], pv[m][:])
                    vsb.append(t)

                # ---- stage 2: psum2[f][p] [h-parity rows j, [g0|g1] w_out]
                p2 = [[None, None], [None, None]]
                for f in range(2):
                    for p in range(2):
                        t = p2pool.tile([128, 512], f32, tag="p2")
                        for k in range(2):
                            lhsT = (
                                vsb[k][:]
                                .rearrange("w (f j q) -> w f j q", f=2, q=2)[:, f, :, p]
                            )
                            nc.tensor.matmul(
                                t[:], lhsT=lhsT, rhs=r2_t[f][:, k, :],
                                start=(k == 0), stop=(k == 1),
                            )
                        p2[f][p] = t

                # ---- odd-parity rows to SBUF (q2c c/d operands + LoLo odd rows)
                odd = []
                for f in range(2):
                    t = opool.tile([128, 512], f32, tag="odd")
                    nc.scalar.copy(t[:], p2[f][1][:])
                    odd.append(t)
                # LoLo even rows to SBUF (DMA cannot read PSUM)
                lolo_e = ypool.tile([128, 256], f32, tag="lolo_e")
                nc.scalar.copy(lolo_e[:], p2[0][0][:, 0:256])

                # ---- LoLo out, rows interleaved (even from copy, odd from odd[0])
                lolo_v = lolo_d[c].rearrange("(j q) w -> j q w", q=2)
                nc.sync.dma_start(out=lolo_v[:, 0, :], in_=lolo_e[:])
                nc.sync.dma_start(out=lolo_v[:, 1, :], in_=odd[0][:, 0:256])

                # ---- q2c. even rows live in p2[f][0] (PSUM), odd in odd[f] (SBUF)
                def ev(f, par):  # w-parity slice of even-h psum, full 512 cols
                    return p2[f][0][:].rearrange("j (c q) -> j c q", q=2)[:, :, par]

                def od(f, par):
                    return odd[f][:].rearrange("j (c q) -> j c q", q=2)[:, :, par]

                # f=1 -> [LoHi | HiHi] sections = bands [0|1] (and [5|4])
                yA = ypool.tile([128, 256], f32, tag="yA")  # Yhr bands 0,1 : a-d
                yB = ypool.tile([128, 256], f32, tag="yB")  # Yhi bands 0,1 : b+c
                yC = ypool.tile([128, 256], f32, tag="yC")  # Yhr bands 5,4 : a+d
                yD = ypool.tile([128, 256], f32, tag="yD")  # Yhi bands 5,4 : b-c
                nc.vector.tensor_sub(out=yA[:], in0=ev(1, 0), in1=od(1, 1))
                nc.vector.tensor_add(out=yB[:], in0=ev(1, 1), in1=od(1, 0))
                nc.vector.tensor_add(out=yC[:], in0=ev(1, 0), in1=od(1, 1))
                nc.vector.tensor_sub(out=yD[:], in0=ev(1, 1), in1=od(1, 0))
                # f=0, HiLo section (cols 128:256 of the w-parity view) = bands 2,3
                zR = ypool.tile([128, 256], f32, tag="zR")  # Yhr bands 2,3 : a-d | a+d
                zI = ypool.tile([128, 256], f32, tag="zI")  # Yhi bands 2,3 : b+c | b-c
                nc.vector.tensor_sub(out=zR[:, 0:128], in0=ev(0, 0)[:, 128:256], in1=od(0, 1)[:, 128:256])
                nc.vector.tensor_add(out=zR[:, 128:256], in0=ev(0, 0)[:, 128:256], in1=od(0, 1)[:, 128:256])
                nc.vector.tensor_add(out=zI[:, 0:128], in0=ev(0, 1)[:, 128:256], in1=od(0, 0)[:, 128:256])
                nc.vector.tensor_sub(out=zI[:, 128:256], in0=ev(0, 1)[:, 128:256], in1=od(0, 0)[:, 128:256])

                # ---- subband DMAs (each [128,128] = 64KB contiguous)
                for tile, dram, bands in (
                    (yA, yhr_d, (0, 1)), (yB, yhi_d, (0, 1)),
                    (yC, yhr_d, (5, 4)), (yD, yhi_d, (5, 4)),
                    (zR, yhr_d, (2, 3)), (zI, yhi_d, (2, 3)),
                ):
                    for s, band in enumerate(bands):
                        nc.sync.dma_start(
                            out=dram[c, band], in_=tile[:, s * 128:(s + 1) * 128]
                        )


_NC_CACHE = {}


def _get_nc(repeat=1):
    if repeat not in _NC_CACHE:
        _NC_CACHE[repeat] = _build_nc(repeat)
    return _NC_CACHE[repeat]


def kernel(X, h0o, h1o):
    from concourse.bass_utils import run_bass_kernel_spmd

    X = np.asarray(X, dtype=np.float32)
    R1, R2a, R2b = _build_consts(np.asarray(h0o), np.asarray(h1o))
    if MM_MODE == "bf16":
        import ml_dtypes
        cast = lambda a: a.astype(ml_dtypes.bfloat16)
        X_in = cast(X)
        R1, R2a, R2b = cast(R1), cast(R2a), cast(R2b)
    else:
        X_in = X

    nc = _get_nc()
    in_maps = [
        {"x": X_in[b], "r1": R1, "r2a": R2a, "r2b": R2b} for b in range(N_CORES)
    ]
    res = run_bass_kernel_spmd(nc, in_maps, core_ids=list(range(N_CORES))).results
    LoLo = np.stack([res[b]["lolo"] for b in range(N_CORES)])
    Yhr = np.stack([res[b]["yhr"] for b in range(N_CORES)])
    Yhi = np.stack([res[b]["yhi"] for b in range(N_CORES)])
    return LoLo, Yhr, Yhi


# revision 6
# speedup vs baseline: 3.0680x; 3.0680x over previous
"""DTCWT level-1 biorthogonal layer (near_sym_a) on 8 Trainium2 NeuronCores.

Math: reference computes, per (batch, channel) 256x256 image X:
    Lo = row_h0(X); Hi = row_h1(X)            (row filter = along W, symmetric pad)
    LoLo = col_h0(Lo)
    LoHi = col_h1(Lo); HiLo = col_h0(Hi); HiHi = col_h1(Hi)   (col = along H)
    q2c on {LoHi, HiHi, HiLo} (scaled by 1/sqrt(2)) -> 6 complex subbands.

Row/col filters commute, so we evaluate col-first:
    V_f = col_hf(X), then <out> = row_hg(V_f).
Both 1-D filters become banded 256x256 matrices (symmetric padding folded in).

Kernel dataflow per image (per core; batch is sharded 1 batch/core):
  stage 1 (PE): psum_v[m][w, :] = sum_h X[h, w+128m] * [A0|A1][h, :]
     - lhsT = X chunk (stationary) => output arrives TRANSPOSED [W, H] for free
     - rhs = concat of both col filter band matrices -> [128, 512] psum
  ACT copy psum_v -> SBUF (v_sb, float32r)
  stage 2 (PE): for f, h-parity p: psum2[f][p] = sum_w V_f[w, 2j+p] * [B|B'][w, :]
     - lhsT = stride-2 columns of v_sb => h-parity split for q2c comes free
     - rhs sections: f=0 -> [LoLo (unscaled) | HiLo*s], f=1 -> [LoHi*s | HiHi*s]
  ACT copies: odd-parity psums -> SBUF, LoLo-even -> SBUF
  DVE q2c: a,b from even psum (direct), c,d from odd SBUF copies; +/- only
  DMA out: LoLo rows interleaved; 12 subband tiles 64KB contiguous each.

Matmuls run in float32r (TF32-ish reduced precision fp32, ~2e-4 rel err).
"""

import numpy as np

MM_MODE = "f32r"  # "f32r" | "f32" | "bf16"

N_CORES = 8
B, C, H, W = 8, 64, 256, 256
NIMG = C  # images per core (one batch per core)


def _conv_mat(h, n):
    """M such that (symmetric-pad correlate1d(x, h)) == M @ x, matching
    jnp.pad(mode='symmetric') + VALID conv_general_dilated (no kernel flip)."""
    L = len(h)
    m = L // 2
    M = np.zeros((n, n), dtype=np.float64)
    for i in range(n):
        for t in range(L):
            j = i + t - m
            if j < 0:
                j = -1 - j
            if j >= n:
                j = 2 * n - 1 - j
            M[i, j] += float(h[t])
    return M


def _build_consts(h0o, h1o):
    A0 = _conv_mat(np.asarray(h0o, dtype=np.float64), H).T  # [h_in, h_out]
    A1 = _conv_mat(np.asarray(h1o, dtype=np.float64), H).T
    s = 1.0 / np.sqrt(2.0)
    R1 = np.concatenate([A0, A1], axis=1)              # [256, 512] col-filter stage
    R2a = np.concatenate([A0, A1 * s], axis=1)         # f=0: [LoLo | HiLo*s]
    R2b = np.concatenate([A0 * s, A1 * s], axis=1)     # f=1: [LoHi*s | HiHi*s]
    f32 = lambda M: np.ascontiguousarray(M.reshape(2, 128, 512).astype(np.float32))
    return f32(R1), f32(R2a), f32(R2b)


def _build_nc(repeat=1):
    import concourse.bass as bass
    import concourse.mybir as mybir
    from concourse import bacc
    from concourse.tile import TileContext

    if MM_MODE == "f32r":
        mdt = mybir.dt.float32r
    elif MM_MODE == "bf16":
        mdt = mybir.dt.bfloat16
    else:
        mdt = mybir.dt.float32
    f32 = mybir.dt.float32

    nc = bacc.Bacc("TRN2", target_bir_lowering=False, debug=False, num_devices=N_CORES)
    x_d = nc.declare_dram_parameter("x", [NIMG, H, W], mdt, isOutput=False)
    r1_d = nc.declare_dram_parameter("r1", [2, 128, 512], mdt, isOutput=False)
    r2a_d = nc.declare_dram_parameter("r2a", [2, 128, 512], mdt, isOutput=False)
    r2b_d = nc.declare_dram_parameter("r2b", [2, 128, 512], mdt, isOutput=False)
    lolo_d = nc.declare_dram_parameter("lolo", [NIMG, H, W], f32, isOutput=True)
    yhr_d = nc.declare_dram_parameter("yhr", [NIMG, 6, 128, 128], f32, isOutput=True)
    yhi_d = nc.declare_dram_parameter("yhi", [NIMG, 6, 128, 128], f32, isOutput=True)

    with TileContext(nc) as tc:
        with (
            tc.tile_pool(name="const", bufs=1) as cpool,
            tc.tile_pool(name="xin", bufs=4) as xpool,
            tc.tile_pool(name="vsb", bufs=4) as vpool,
            tc.tile_pool(name="osb", bufs=4) as opool,
            tc.tile_pool(name="yout", bufs=14) as ypool,
            tc.tile_pool(name="pv", bufs=4, space="PSUM") as pvpool,
            tc.tile_pool(name="p2", bufs=4, space="PSUM") as p2pool,
        ):
            r1_t = cpool.tile([128, 2, 512], mdt)
            r2a_t = cpool.tile([128, 2, 512], mdt)
            r2b_t = cpool.tile([128, 2, 512], mdt)
            for k in range(2):
                nc.sync.dma_start(out=r1_t[:, k, :], in_=r1_d[k])
                nc.sync.dma_start(out=r2a_t[:, k, :], in_=r2a_d[k])
                nc.sync.dma_start(out=r2b_t[:, k, :], in_=r2b_d[k])
            r2_t = [r2a_t, r2b_t]

            with tc.For_i(0, repeat, 1):
                _image_loop(nc, tc, bass, mybir, mdt, f32,
                            x_d, lolo_d, yhr_d, yhi_d, r1_t, r2_t,
                            xpool, vpool, opool, ypool, pvpool, p2pool)
    nc.compile()
    return nc


def _image_loop(nc, tc, bass, mybir, mdt, f32, x_d, lolo_d, yhr_d, yhi_d,
                r1_t, r2_t, xpool, vpool, opool, ypool, pvpool, p2pool):
    if True:
            for c in range(NIMG):
                # ---- load image c in ONE dma: out [p, k, w] <- dram (k p) w
                xt = xpool.tile([128, 2, W], mdt, tag="x")
                nc.sync.dma_start(
                    out=xt[:],
                    in_=x_d[c].rearrange("(k p) w -> p k w", k=2),
                )

                # ---- stage 1: psum_v[m] [w_local, [V0|V1] h_out] for W chunk m
                pv = []
                for m in range(2):
                    t = pvpool.tile([128, 512], f32, tag="pv")
                    for k in range(2):
                        nc.tensor.matmul(
                            t[:], lhsT=xt[:, k, m * 128:(m + 1) * 128],
                            rhs=r1_t[:, k, :], start=(k == 0), stop=(k == 1),
                        )
                    pv.append(t)

                # ---- V to SBUF (rounded to matmul dtype for stage-2 lhsT)
                vsb = []
                for m in range(2):
                    t = vpool.tile([128, 512], mdt, tag="v")
                    nc.scalar.copy(t[:], pv[m][:])
                    vsb.append(t)

                # ---- stage 2: psum2[f][p] [h-parity rows j, [g0|g1] w_out]
                p2 = [[None, None], [None, None]]
                for f in range(2):
                    for p in range(2):
                        t = p2pool.tile([128, 512], f32, tag="p2")
                        for k in range(2):
                            lhsT = (
                                vsb[k][:]
                                .rearrange("w (f j q) -> w f j q", f=2, q=2)[:, f, :, p]
                            )
                            nc.tensor.matmul(
                                t[:], lhsT=lhsT, rhs=r2_t[f][:, k, :],
                                start=(k == 0), stop=(k == 1),
                            )
                        p2[f][p] = t

                # ---- PSUM -> SBUF copies (ACT). LoLo parities land interleavable
                # in one tile; HiLo-odd and f=1-odd feed q2c.
                lolo_sb = ypool.tile([128, 2, 256], f32, tag="lolo")  # [j, parity, w]
                nc.scalar.copy(lolo_sb[:, 0, :], p2[0][0][:, 0:256])
                nc.scalar.copy(lolo_sb[:, 1, :], p2[0][1][:, 0:256])
                odd0h = opool.tile([128, 256], f32, tag="odd0h")      # HiLo odd rows
                nc.scalar.copy(odd0h[:], p2[0][1][:, 256:512])
                odd1 = opool.tile([128, 512], f32, tag="odd1")        # [LoHi|HiHi] odd
                nc.scalar.copy(odd1[:], p2[1][1][:])

                # ---- LoLo out in ONE dma: dram (j q) w <- [j, q, w]
                nc.sync.dma_start(
                    out=lolo_d[c].rearrange("(j q) w -> j q w", q=2),
                    in_=lolo_sb[:],
                )

                # ---- q2c into band-ordered mega-tiles [j, band, i]
                def ev(f, par):  # w-parity slice of even-h psum, full 512 cols
                    return p2[f][0][:].rearrange("j (c q) -> j c q", q=2)[:, :, par]

                def od1(par):
                    return odd1[:].rearrange("j (c q) -> j c q", q=2)[:, :, par]

                def od0(par):
                    return odd0h[:].rearrange("j (c q) -> j c q", q=2)[:, :, par]

                yr = ypool.tile([128, 6, 128], f32, tag="yr")
                yi = ypool.tile([128, 6, 128], f32, tag="yi")
                yrf = yr[:].rearrange("j b i -> j (b i)")
                yif = yi[:].rearrange("j b i -> j (b i)")
                # bands 0,1 = [LoHi|HiHi] a-d / b+c (one [128,256] op each)
                nc.vector.tensor_sub(out=yrf[:, 0:256], in0=ev(1, 0), in1=od1(1))
                nc.vector.tensor_add(out=yif[:, 0:256], in0=ev(1, 1), in1=od1(0))
                # bands 2,3 = HiLo a-d | a+d (HiLo = cols 128:256 of f=0 views)
                e0a, e0b = ev(0, 0)[:, 128:256], ev(0, 1)[:, 128:256]
                nc.vector.tensor_sub(out=yr[:, 2, :], in0=e0a, in1=od0(1))
                nc.vector.tensor_add(out=yr[:, 3, :], in0=e0a, in1=od0(1))
                nc.vector.tensor_add(out=yi[:, 2, :], in0=e0b, in1=od0(0))
                nc.vector.tensor_sub(out=yi[:, 3, :], in0=e0b, in1=od0(0))
                # band 4 = HiHi a+d / b-c ; band 5 = LoHi a+d / b-c
                nc.vector.tensor_add(out=yr[:, 4, :], in0=ev(1, 0)[:, 128:256], in1=od1(1)[:, 128:256])
                nc.vector.tensor_add(out=yr[:, 5, :], in0=ev(1, 0)[:, 0:128], in1=od1(1)[:, 0:128])
                nc.vector.tensor_sub(out=yi[:, 4, :], in0=ev(1, 1)[:, 128:256], in1=od1(0)[:, 128:256])
                nc.vector.tensor_sub(out=yi[:, 5, :], in0=ev(1, 1)[:, 0:128], in1=od1(0)[:, 0:128])

                # ---- one dma per output tensor: dram [band, j, i] <- [j, band, i]
                nc.gpsimd.dma_start(
                    out=yhr_d[c].rearrange("b j i -> j b i"), in_=yr[:]
                )
                nc.gpsimd.dma_start(
                    out=yhi_d[c].rearrange("b j i -> j b i"), in_=yi[:]
                )


_NC_CACHE = {}


def _get_nc(repeat=1):
    if repeat not in _NC_CACHE:
        _NC_CACHE[repeat] = _build_nc(repeat)
    return _NC_CACHE[repeat]


def kernel(X, h0o, h1o):
    from concourse.bass_utils import run_bass_kernel_spmd

    X = np.asarray(X, dtype=np.float32)
    R1, R2a, R2b = _build_consts(np.asarray(h0o), np.asarray(h1o))
    if MM_MODE == "bf16":
        import ml_dtypes
        cast = lambda a: a.astype(ml_dtypes.bfloat16)
        X_in = cast(X)
        R1, R2a, R2b = cast(R1), cast(R2a), cast(R2b)
    else:
        X_in = X

    nc = _get_nc()
    in_maps = [
        {"x": X_in[b], "r1": R1, "r2a": R2a, "r2b": R2b} for b in range(N_CORES)
    ]
    res = run_bass_kernel_spmd(nc, in_maps, core_ids=list(range(N_CORES))).results
    LoLo = np.stack([res[b]["lolo"] for b in range(N_CORES)])
    Yhr = np.stack([res[b]["yhr"] for b in range(N_CORES)])
    Yhi = np.stack([res[b]["yhi"] for b in range(N_CORES)])
    return LoLo, Yhr, Yhi


# revision 10
# speedup vs baseline: 4.6332x; 1.5102x over previous
"""DTCWT level-1 biorthogonal layer (near_sym_a) on 8 Trainium2 NeuronCores.

Math: reference computes, per (batch, channel) 256x256 image X:
    Lo = row_h0(X); Hi = row_h1(X)            (row filter = along W, symmetric pad)
    LoLo = col_h0(Lo)
    LoHi = col_h1(Lo); HiLo = col_h0(Hi); HiHi = col_h1(Hi)   (col = along H)
    q2c on {LoHi, HiHi, HiLo} (scaled by 1/sqrt(2)) -> 6 complex subbands.

Row/col filters commute, so we evaluate col-first:
    V_f = col_hf(X), then <out> = row_hg(V_f).
Both 1-D filters become banded 256x256 matrices (symmetric padding folded in).

Kernel dataflow per image (per core; batch is sharded 1 batch/core):
  stage 1 (PE): psum_v[m][w, :] = sum_h X[h, w+128m] * [A0|A1][h, :]
     - lhsT = X chunk (stationary) => output arrives TRANSPOSED [W, H] for free
     - rhs = concat of both col filter band matrices -> [128, 512] psum
  ACT copy psum_v -> SBUF (v_sb, float32r)
  stage 2 (PE): for f, h-parity p: psum2[f][p] = sum_w V_f[w, 2j+p] * [B|B'][w, :]
     - lhsT = stride-2 columns of v_sb => h-parity split for q2c comes free
     - rhs sections: f=0 -> [LoLo (unscaled) | HiLo*s], f=1 -> [LoHi*s | HiHi*s]
  ACT copies: odd-parity psums -> SBUF, LoLo-even -> SBUF
  DVE q2c: a,b from even psum (direct), c,d from odd SBUF copies; +/- only
  DMA out: LoLo rows interleaved; 12 subband tiles 64KB contiguous each.

Matmuls run in float32r (TF32-ish reduced precision fp32, ~2e-4 rel err).
"""

import numpy as np

MM_MODE = "f32r"  # "f32r" | "f32" | "bf16"

N_CORES = 8
B, C, H, W = 8, 64, 256, 256
NIMG = C  # images per core (one batch per core)


def _conv_mat(h, n):
    """M such that (symmetric-pad correlate1d(x, h)) == M @ x, matching
    jnp.pad(mode='symmetric') + VALID conv_general_dilated (no kernel flip)."""
    L = len(h)
    m = L // 2
    M = np.zeros((n, n), dtype=np.float64)
    for i in range(n):
        for t in range(L):
            j = i + t - m
            if j < 0:
                j = -1 - j
            if j >= n:
                j = 2 * n - 1 - j
            M[i, j] += float(h[t])
    return M


def _build_consts(h0o, h1o):
    A0 = _conv_mat(np.asarray(h0o, dtype=np.float64), H).T  # [h_in, h_out]
    A1 = _conv_mat(np.asarray(h1o, dtype=np.float64), H).T
    s = 1.0 / np.sqrt(2.0)
    R1 = np.concatenate([A0, A1], axis=1)              # [256, 512] col-filter stage
    R2a = np.concatenate([A0, A1 * s], axis=1)         # f=0: [LoLo | HiLo*s]
    R2b = np.concatenate([A0 * s, A1 * s], axis=1)     # f=1: [LoHi*s | HiHi*s]
    f32 = lambda M: np.ascontiguousarray(M.reshape(2, 128, 512).astype(np.float32))
    return f32(R1), f32(R2a), f32(R2b)


def _build_nc(repeat=1):
    import concourse.bass as bass
    import concourse.mybir as mybir
    from concourse import bacc
    from concourse.tile import TileContext

    if MM_MODE == "f32r":
        mdt = mybir.dt.float32r
    elif MM_MODE == "bf16":
        mdt = mybir.dt.bfloat16
    else:
        mdt = mybir.dt.float32
    f32 = mybir.dt.float32

    nc = bacc.Bacc("TRN2", target_bir_lowering=False, debug=False, num_devices=N_CORES)
    x_d = nc.declare_dram_parameter("x", [NIMG, H, W], mdt, isOutput=False)
    r1_d = nc.declare_dram_parameter("r1", [2, 128, 512], mdt, isOutput=False)
    r2a_d = nc.declare_dram_parameter("r2a", [2, 128, 512], mdt, isOutput=False)
    r2b_d = nc.declare_dram_parameter("r2b", [2, 128, 512], mdt, isOutput=False)
    lolo_d = nc.declare_dram_parameter("lolo", [NIMG, H, W], f32, isOutput=True)
    yhr_d = nc.declare_dram_parameter("yhr", [NIMG, 6, 128, 128], f32, isOutput=True)
    yhi_d = nc.declare_dram_parameter("yhi", [NIMG, 6, 128, 128], f32, isOutput=True)

    with TileContext(nc) as tc:
        with (
            tc.tile_pool(name="const", bufs=1) as cpool,
            tc.tile_pool(name="xin", bufs=4) as xpool,
            tc.tile_pool(name="vsb", bufs=4) as vpool,
            tc.tile_pool(name="osb", bufs=4) as opool,
            tc.tile_pool(name="yout", bufs=14) as ypool,
            tc.tile_pool(name="pv", bufs=2, space="PSUM") as pvpool,
            tc.tile_pool(name="p2e", bufs=4, space="PSUM") as p2epool,
            tc.tile_pool(name="p2o", bufs=2, space="PSUM") as p2opool,
        ):
            r1_t = cpool.tile([128, 2, 512], mdt)
            r2a_t = cpool.tile([128, 2, 512], mdt)
            r2b_t = cpool.tile([128, 2, 512], mdt)
            for k in range(2):
                nc.sync.dma_start(out=r1_t[:, k, :], in_=r1_d[k])
                nc.sync.dma_start(out=r2a_t[:, k, :], in_=r2a_d[k])
                nc.sync.dma_start(out=r2b_t[:, k, :], in_=r2b_d[k])
            r2_t = [r2a_t, r2b_t]

            with tc.For_i(0, repeat, 1):
                _image_loop(nc, tc, bass, mybir, mdt, f32,
                            x_d, lolo_d, yhr_d, yhi_d, r1_t, r2_t,
                            xpool, vpool, opool, ypool, pvpool, p2epool, p2opool)
    nc.compile()
    return nc


def _image_loop(nc, tc, bass, mybir, mdt, f32, x_d, lolo_d, yhr_d, yhi_d,
                r1_t, r2_t, xpool, vpool, opool, ypool, pvpool, p2epool, p2opool):
    if True:
            for c in range(NIMG):
                # ---- load image c in ONE dma: out [p, k, w] <- dram (k p) w
                xt = xpool.tile([128, 2, W], mdt, tag="x")
                nc.sync.dma_start(
                    out=xt[:],
                    in_=x_d[c].rearrange("(k p) w -> p k w", k=2),
                )

                # ---- stage 1: psum_v[m] [w_local, [V0|V1] h_out] for W chunk m
                pv = []
                for m in range(2):
                    t = pvpool.tile([128, 512], f32, tag="pv")
                    for k in range(2):
                        nc.tensor.matmul(
                            t[:], lhsT=xt[:, k, m * 128:(m + 1) * 128],
                            rhs=r1_t[:, k, :], start=(k == 0), stop=(k == 1),
                        )
                    pv.append(t)

                # ---- V to SBUF (rounded to matmul dtype for stage-2 lhsT)
                vsb = []
                for m in range(2):
                    t = vpool.tile([128, 512], mdt, tag="v")
                    nc.scalar.copy(t[:], pv[m][:])
                    vsb.append(t)

                # ---- stage 2: psum2[f][p] [h-parity rows j, [g0|g1] w_out]
                # separate tags for even/odd parity: odd tiles are released
                # right after their ACT copies, deepening the pipeline
                p2 = [[None, None], [None, None]]
                for f in range(2):
                    for p in range(2):
                        pool_ = p2epool if p == 0 else p2opool
                        t = pool_.tile([128, 512], f32, tag=f"p2{'eo'[p]}")
                        for k in range(2):
                            lhsT = (
                                vsb[k][:]
                                .rearrange("w (f j q) -> w f j q", f=2, q=2)[:, f, :, p]
                            )
                            nc.tensor.matmul(
                                t[:], lhsT=lhsT, rhs=r2_t[f][:, k, :],
                                start=(k == 0), stop=(k == 1),
                            )
                        p2[f][p] = t

                # ---- PSUM -> SBUF copies (ACT). LoLo parities land interleavable
                # in one tile; HiLo-odd and f=1-odd feed q2c.
                lolo_sb = ypool.tile([128, 2, 256], f32, tag="lolo")  # [j, parity, w]
                nc.scalar.copy(lolo_sb[:, 0, :], p2[0][0][:, 0:256])
                nc.scalar.copy(lolo_sb[:, 1, :], p2[0][1][:, 0:256])
                odd0h = opool.tile([128, 256], f32, tag="odd0h")      # HiLo odd rows
                nc.scalar.copy(odd0h[:], p2[0][1][:, 256:512])
                odd1 = opool.tile([128, 512], f32, tag="odd1")        # [LoHi|HiHi] odd
                nc.scalar.copy(odd1[:], p2[1][1][:])

                # ---- LoLo out in ONE dma: dram (j q) w <- [j, q, w]
                nc.sync.dma_start(
                    out=lolo_d[c].rearrange("(j q) w -> j q w", q=2),
                    in_=lolo_sb[:],
                )

                # ---- q2c into band-ordered mega-tiles [j, band, i]
                def ev(f, par):  # w-parity slice of even-h psum, full 512 cols
                    return p2[f][0][:].rearrange("j (c q) -> j c q", q=2)[:, :, par]

                def od1(par):
                    return odd1[:].rearrange("j (c q) -> j c q", q=2)[:, :, par]

                def od0(par):
                    return odd0h[:].rearrange("j (c q) -> j c q", q=2)[:, :, par]

                yr = ypool.tile([128, 6, 128], f32, tag="yr")
                yi = ypool.tile([128, 6, 128], f32, tag="yi")
                yrf = yr[:].rearrange("j b i -> j (b i)")
                yif = yi[:].rearrange("j b i -> j (b i)")
                # bands 0,1 = [LoHi|HiHi] a-d / b+c (one [128,256] op each)
                nc.vector.tensor_sub(out=yrf[:, 0:256], in0=ev(1, 0), in1=od1(1))
                nc.vector.tensor_add(out=yif[:, 0:256], in0=ev(1, 1), in1=od1(0))
                # bands 2,3 = HiLo a-d | a+d (HiLo = cols 128:256 of f=0 views)
                e0a, e0b = ev(0, 0)[:, 128:256], ev(0, 1)[:, 128:256]
                nc.vector.tensor_sub(out=yr[:, 2, :], in0=e0a, in1=od0(1))
                nc.vector.tensor_add(out=yr[:, 3, :], in0=e0a, in1=od0(1))
                nc.vector.tensor_add(out=yi[:, 2, :], in0=e0b, in1=od0(0))
                nc.vector.tensor_sub(out=yi[:, 3, :], in0=e0b, in1=od0(0))
                # band 4 = HiHi a+d / b-c ; band 5 = LoHi a+d / b-c
                nc.vector.tensor_add(out=yr[:, 4, :], in0=ev(1, 0)[:, 128:256], in1=od1(1)[:, 128:256])
                nc.vector.tensor_add(out=yr[:, 5, :], in0=ev(1, 0)[:, 0:128], in1=od1(1)[:, 0:128])
                nc.vector.tensor_sub(out=yi[:, 4, :], in0=ev(1, 1)[:, 128:256], in1=od1(0)[:, 128:256])
                nc.vector.tensor_sub(out=yi[:, 5, :], in0=ev(1, 1)[:, 0:128], in1=od1(0)[:, 0:128])

                # ---- one dma per output tensor: dram [band, j, i] <- [j, band, i]
                # Spread across rings: SP (in+lolo), ACT HWDGE (yhr), SWDGE (yhi)
                nc.scalar.dma_start(
                    out=yhr_d[c].rearrange("b j i -> j b i"), in_=yr[:]
                )
                nc.gpsimd.dma_start(
                    out=yhi_d[c].rearrange("b j i -> j b i"), in_=yi[:]
                )


_NC_CACHE = {}


def _get_nc(repeat=1):
    if repeat not in _NC_CACHE:
        _NC_CACHE[repeat] = _build_nc(repeat)
    return _NC_CACHE[repeat]


def kernel(X, h0o, h1o):
    from concourse.bass_utils import run_bass_kernel_spmd

    X = np.asarray(X, dtype=np.float32)
    R1, R2a, R2b = _build_consts(np.asarray(h0o), np.asarray(h1o))
    if MM_MODE == "bf16":
        import ml_dtypes
        cast = lambda a: a.astype(ml_dtypes.bfloat16)
        X_in = cast(X)
        R1, R2a, R2b = cast(R1), cast(R2a), cast(R2b)
    else:
        X_in = X

    nc = _get_nc()
    in_maps = [
        {"x": X_in[b], "r1": R1, "r2a": R2a, "r2b": R2b} for b in range(N_CORES)
    ]
    res = run_bass_kernel_spmd(nc, in_maps, core_ids=list(range(N_CORES))).results
    LoLo = np.stack([res[b]["lolo"] for b in range(N_CORES)])
    Yhr = np.stack([res[b]["yhr"] for b in range(N_CORES)])
    Yhi = np.stack([res[b]["yhi"] for b in range(N_CORES)])
    return LoLo, Yhr, Yhi
